# revision 14
# baseline (speedup 1.0000x reference)
"""ConvHex GNN message-passing kernel for Trainium2 (8 NeuronCores).

Math (per batch b):
    out[b,o,h] = ( Wc[o,:] @ x[b,:,h]
                   + sum_k Wn[o,:,k] @ x[b,:,idx[h,k]]*valid ) / nu + bias[o]

Strategy (V4):
  - Data parallel over batch: 32 batch elems / core.
  - Neighbor gather via SWDGE dma_gather(transpose=True) from an HBM-resident
    fp16 token table xtab[h] = full 4KB (b,c) feature column (32 batches).
    The X-bar transpose delivers tokens in compute layout: partitions
    (b%2)*64+c, free (pair, dest-slot).  One index per (tap, dest pixel).
  - Ring-safe calls: 128 idxs padded / 112 live (114 descs < 128-deep ring).
  - Center tap via direct chunked DMA of x; 7 PSUM-accumulated fp16 matmuls
    per (quad-pair, chunk) at N=512 with block-diag [[W.T,0],[0,W.T]]
    weights (scaled 1/nu host-side). Invalid neighbors -> zero row at H.
  - fp16 output, host converts/reassembles.
"""

import numpy as np

import concourse.bacc as bacc
import concourse.mybir as mybir
import concourse.tile as tile
from concourse import bass_utils

B, C, H, K = 256, 64, 1855, 6
NCORES = 8
BL = B // NCORES          # 32
NPAIR = BL // 2           # 16
HP = H + 1                # zero row at H
P = 128
ELEM = 2048               # fp16 elems per token = 32 batches x 64 ch
NI = 128                  # static idxs per gather call
LIVE = 112                # live idxs per call (ring: 112*16/16+2=114<128)
NCHUNK = (H + LIVE - 1) // LIVE               # 17
CH = [(c * LIVE, min(LIVE, H - c * LIVE)) for c in range(NCHUNK)]
HPAD = NCHUNK * LIVE      # 1904 padded H in the output buffer
GRP = 4                   # output chunks per store DMA
NGRP = (NCHUNK + GRP - 1) // GRP              # 5
NQ = 4

_F32 = mybir.dt.float32
_F16 = mybir.dt.float16
_I16 = mybir.dt.int16


def _host_prep(x, neighbors, weight_center, weight_neighbors, bias):
    x = np.asarray(x, dtype=np.float32)
    neighbors = np.asarray(neighbors)
    wc = np.asarray(weight_center, dtype=np.float32)
    wn = np.asarray(weight_neighbors, dtype=np.float32)
    bias = np.asarray(bias, dtype=np.float32)

    nu = np.float32((neighbors[0] >= 0).sum() + 1)
    safe = np.where(neighbors >= 0, neighbors, H).astype(np.int16)  # [H,K]

    # Index tables: per (tap, chunk) a [128, NI/16] wrapped block.
    idx_pack = np.full((K, NCHUNK, NI), -1, dtype=np.int16)
    for k in range(K):
        for ci, (c0, n) in enumerate(CH):
            idx_pack[k, ci, :n] = safe[c0:c0 + n, k]
    w = idx_pack.reshape(K, NCHUNK, NI // 16, 16)
    idx_w = np.tile(w.transpose(3, 0, 1, 2), (8, 1, 1, 1))
    idx_w = np.ascontiguousarray(idx_w.reshape(P, K * NCHUNK * (NI // 16)))

    x16 = x.astype(np.float16)                      # [B, C, H]
    # token table: xtab[core, h, e], e = (b%32)*64 + c
    xtab = np.zeros((NCORES, HP, ELEM), dtype=np.float16)
    xs = x16.reshape(NCORES, BL, C, H)
    xtab[:, :H, :] = xs.transpose(0, 3, 1, 2).reshape(NCORES, H, ELEM)

    # center operand, chunked quad-pair layout:
    # x_all[core, (b%2)*64+c, ci, qd*512 + q*NI + j] = x[b, c, c0+j]
    # with pair = b//2 = qd*4+q.
    x_all = np.zeros((NCORES, P, NCHUNK, 4, 4, NI), dtype=np.float16)
    xp = x16.reshape(NCORES, NPAIR, 2, C, H)
    for ci, (c0, n) in enumerate(CH):
        blk = xp[:, :, :, :, c0:c0 + n]              # [core,pair,bp,c,n]
        blk = blk.transpose(0, 2, 3, 1, 4).reshape(NCORES, P, 4, 4, n)
        x_all[:, :, ci, :, :, :n] = blk
    x_all = np.ascontiguousarray(x_all.reshape(NCORES, P, NCHUNK, 4 * 4 * NI))

    # fp16 block-diag weights / nu, packed [128, 7*128]
    w_all = np.zeros((K + 1, P, P), dtype=np.float16)
    mats = [wc] + [wn[:, :, k] for k in range(K)]
    for s, wmat in enumerate(mats):
        wt = (wmat.T / nu).astype(np.float16)
        w_all[s, :C, :C] = wt
        w_all[s, C:, C:] = wt
    w_pack = np.ascontiguousarray(w_all.transpose(1, 0, 2).reshape(P, (K + 1) * P))

    bias2 = np.concatenate([bias, bias]).reshape(P, 1).astype(np.float32)
    return xtab, x_all, idx_w, w_pack, bias2


def _build_program(idx_w, w_pack, bias2):
    nc = bacc.Bacc("TRN2", target_bir_lowering=False, debug=False,
                   num_devices=NCORES, num_swdge_queues=NQ,
                   enable_asserts=False)

    xtab_d = nc.dram_tensor("xtab", [HP, ELEM], _F16, kind="ExternalInput")
    xall_d = nc.dram_tensor("xall", [P, NCHUNK, 4 * 4 * NI], _F16,
                            kind="ExternalInput")
    out_d = nc.dram_tensor("out", [NPAIR, P, HPAD], _F16, kind="ExternalOutput")

    idx_dram = nc.inline_tensor(idx_w, name="idx_w")
    w_dram = nc.inline_tensor(w_pack, name="w_pack")
    b_dram = nc.inline_tensor(bias2, name="bias2")

    call_no = 0
    with tile.TileContext(nc) as tc:
        with (
            tc.tile_pool(name="consts", bufs=1) as cpool,
            tc.tile_pool(name="gp", bufs=20) as gpool,
            tc.tile_pool(name="op", bufs=20) as opool,
            tc.tile_pool(name="ps", bufs=8, space="PSUM") as pspool,
        ):
            idx_sb = cpool.tile([P, K * NCHUNK * (NI // 16)], _I16)
            nc.sync.dma_start(idx_sb[:], idx_dram[:])
            w_sb = cpool.tile([P, K + 1, P], _F16)
            nc.sync.dma_start(w_sb[:], w_dram[:])
            b_sb = cpool.tile([P, 1], _F32)
            nc.sync.dma_start(b_sb[:], b_dram[:])
            x_sb = cpool.tile([P, NCHUNK, 4 * 4 * NI], _F16)

            o_ts = {}
            for ci, (c0, n) in enumerate(CH):
                # center operand slab for this chunk, loaded just-in-time
                nc.sync.dma_start(x_sb[:, ci, :], xall_d[:, ci, :])
                g_ts = []
                for k in range(K):
                    g_t = gpool.tile([P, NPAIR, NI], _F16)
                    io = (k * NCHUNK + ci) * (NI // 16)
                    nc.gpsimd.dma_gather(
                        g_t[:], xtab_d[:], idx_sb[:, io:io + NI // 16],
                        num_idxs=NI, num_idxs_reg=n,
                        elem_size=ELEM, transpose=True,
                        queue_num=call_no % NQ)
                    call_no += 1
                    g_ts.append(g_t)
                gi = ci % GRP          # position within output group
                for qd in range(4):
                    ps = pspool.tile([P, 4 * NI], _F32)
                    nc.tensor.matmul(
                        ps[:, :], w_sb[:, 0, :],
                        x_sb[:, ci, qd * 4 * NI:(qd + 1) * 4 * NI],
                        start=True, stop=False)
                    for k in range(K):
                        nc.tensor.matmul(
                            ps[:, :], w_sb[:, k + 1, :],
                            g_ts[k][:, qd * 4:qd * 4 + 4, :],
                            start=False, stop=(k == K - 1))
                    for q in range(4):
                        pair = qd * 4 + q
                        if gi == 0:
                            o_ts[pair] = opool.tile([P, GRP * LIVE], _F16,
                                                    name="o_t", tag="o_t")
                        nc.vector.tensor_scalar_add(
                            o_ts[pair][:, gi * LIVE:gi * LIVE + n],
                            ps[:, q * NI:q * NI + n], b_sb[:, :1])
                if gi == GRP - 1 or ci == NCHUNK - 1:
                    g0 = (ci // GRP) * GRP * LIVE
                    gw = (gi + 1) * LIVE
                    for pair in range(NPAIR):
                        nc.sync.dma_start(
                            out_d[pair, :, g0:g0 + gw],
                            o_ts[pair][:, :gw])

    nc.compile()
    return nc


def _run(inputs, trace=False):
    xtab, x_all, idx_w, w_pack, bias2 = _host_prep(
        inputs["x"], inputs["neighbors"], inputs["weight_center"],
        inputs["weight_neighbors"], inputs["bias"])
    nc = _build_program(idx_w, w_pack, bias2)
    in_maps = [{"xtab": xtab[c], "xall": x_all[c]} for c in range(NCORES)]
    res = bass_utils.run_bass_kernel_spmd(
        nc, in_maps, core_ids=list(range(NCORES)), trace=trace)
    outs = np.stack([r["out"] for r in res.results])  # [NC, NPAIR, 128, HPAD]
    outs = outs[:, :, :, :H].astype(np.float32)
    outs = outs.reshape(NCORES, NPAIR, 2, C, H).reshape(B, C, H)
    return np.ascontiguousarray(outs), res


def kernel(x, neighbors, weight_center, weight_neighbors, bias):
    out, _ = _run(dict(x=x, neighbors=neighbors, weight_center=weight_center,
                       weight_neighbors=weight_neighbors, bias=bias))
    return out


# revision 15
# speedup vs baseline: 1.0282x; 1.0282x over previous
"""ConvHex GNN message-passing kernel for Trainium2 (8 NeuronCores).

Math (per batch b):
    out[b,o,h] = ( Wc[o,:] @ x[b,:,h]
                   + sum_k Wn[o,:,k] @ x[b,:,idx[h,k]]*valid ) / nu + bias[o]

Strategy (V4):
  - Data parallel over batch: 32 batch elems / core.
  - Neighbor gather via SWDGE dma_gather(transpose=True) from an HBM-resident
    fp16 token table xtab[h] = full 4KB (b,c) feature column (32 batches).
    The X-bar transpose delivers tokens in compute layout: partitions
    (b%2)*64+c, free (pair, dest-slot).  One index per (tap, dest pixel).
  - Ring-safe calls: 128 idxs padded / 112 live (114 descs < 128-deep ring).
  - Center tap via direct chunked DMA of x; 7 PSUM-accumulated fp16 matmuls
    per (quad-pair, chunk) at N=512 with block-diag [[W.T,0],[0,W.T]]
    weights (scaled 1/nu host-side). Invalid neighbors -> zero row at H.
  - fp16 output, host converts/reassembles.
"""

import numpy as np

import concourse.bacc as bacc
import concourse.mybir as mybir
import concourse.tile as tile
from concourse import bass_utils

B, C, H, K = 256, 64, 1855, 6
NCORES = 8
BL = B // NCORES          # 32
NPAIR = BL // 2           # 16
HP = H + 1                # zero row at H
P = 128
ELEM = 2048               # fp16 elems per token = 32 batches x 64 ch
NI = 128                  # static idxs per gather call
LIVE = 112                # live idxs per call (ring: 112*16/16+2=114<128)
NCHUNK = (H + LIVE - 1) // LIVE               # 17
CH = [(c * LIVE, min(LIVE, H - c * LIVE)) for c in range(NCHUNK)]
HPAD = NCHUNK * LIVE      # 1904 padded H in the output buffer
GRP = 4                   # output chunks per store DMA
NGRP = (NCHUNK + GRP - 1) // GRP              # 5
NQ = 4

_F32 = mybir.dt.float32
_F16 = mybir.dt.float16
_I16 = mybir.dt.int16


def _host_prep(x, neighbors, weight_center, weight_neighbors, bias):
    x = np.asarray(x, dtype=np.float32)
    neighbors = np.asarray(neighbors)
    wc = np.asarray(weight_center, dtype=np.float32)
    wn = np.asarray(weight_neighbors, dtype=np.float32)
    bias = np.asarray(bias, dtype=np.float32)

    nu = np.float32((neighbors[0] >= 0).sum() + 1)
    safe = np.where(neighbors >= 0, neighbors, H).astype(np.int16)  # [H,K]

    # Index tables: per (tap, chunk) a [128, NI/16] wrapped block.
    idx_pack = np.full((K, NCHUNK, NI), -1, dtype=np.int16)
    for k in range(K):
        for ci, (c0, n) in enumerate(CH):
            idx_pack[k, ci, :n] = safe[c0:c0 + n, k]
    w = idx_pack.reshape(K, NCHUNK, NI // 16, 16)
    idx_w = np.tile(w.transpose(3, 0, 1, 2), (8, 1, 1, 1))
    idx_w = np.ascontiguousarray(idx_w.reshape(P, K * NCHUNK * (NI // 16)))

    x16 = x.astype(np.float16)                      # [B, C, H]
    # token table: xtab[core, h, e], e = (b%32)*64 + c
    xtab = np.zeros((NCORES, HP, ELEM), dtype=np.float16)
    xs = x16.reshape(NCORES, BL, C, H)
    xtab[:, :H, :] = xs.transpose(0, 3, 1, 2).reshape(NCORES, H, ELEM)

    # center operand, chunked quad-pair layout:
    # x_all[core, (b%2)*64+c, ci, qd*512 + q*NI + j] = x[b, c, c0+j]
    # with pair = b//2 = qd*4+q.
    x_all = np.zeros((NCORES, P, NCHUNK, 4, 4, NI), dtype=np.float16)
    xp = x16.reshape(NCORES, NPAIR, 2, C, H)
    for ci, (c0, n) in enumerate(CH):
        blk = xp[:, :, :, :, c0:c0 + n]              # [core,pair,bp,c,n]
        blk = blk.transpose(0, 2, 3, 1, 4).reshape(NCORES, P, 4, 4, n)
        x_all[:, :, ci, :, :, :n] = blk
    x_all = np.ascontiguousarray(x_all.reshape(NCORES, P, NCHUNK, 4 * 4 * NI))

    # fp16 block-diag weights / nu, packed [128, 7*128]
    w_all = np.zeros((K + 1, P, P), dtype=np.float16)
    mats = [wc] + [wn[:, :, k] for k in range(K)]
    for s, wmat in enumerate(mats):
        wt = (wmat.T / nu).astype(np.float16)
        w_all[s, :C, :C] = wt
        w_all[s, C:, C:] = wt
    w_pack = np.ascontiguousarray(w_all.transpose(1, 0, 2).reshape(P, (K + 1) * P))

    bias2 = np.concatenate([bias, bias]).reshape(P, 1).astype(np.float32)
    return xtab, x_all, idx_w, w_pack, bias2


def _build_program(idx_w, w_pack, bias2):
    nc = bacc.Bacc("TRN2", target_bir_lowering=False, debug=False,
                   num_devices=NCORES, num_swdge_queues=NQ,
                   enable_asserts=False)

    xtab_d = nc.dram_tensor("xtab", [HP, ELEM], _F16, kind="ExternalInput")
    xall_d = nc.dram_tensor("xall", [P, NCHUNK, 4 * 4 * NI], _F16,
                            kind="ExternalInput")
    out_d = nc.dram_tensor("out", [NPAIR, P, HPAD], _F16, kind="ExternalOutput")

    idx_dram = nc.inline_tensor(idx_w, name="idx_w")
    w_dram = nc.inline_tensor(w_pack, name="w_pack")
    b_dram = nc.inline_tensor(bias2, name="bias2")

    call_no = 0
    with tile.TileContext(nc) as tc:
        with (
            tc.tile_pool(name="consts", bufs=1) as cpool,
            tc.tile_pool(name="gp", bufs=14) as gpool,
            tc.tile_pool(name="op", bufs=20) as opool,
            tc.tile_pool(name="ps", bufs=8, space="PSUM") as pspool,
        ):
            idx_sb = cpool.tile([P, K * NCHUNK * (NI // 16)], _I16)
            nc.sync.dma_start(idx_sb[:], idx_dram[:])
            w_sb = cpool.tile([P, K + 1, P], _F16)
            nc.sync.dma_start(w_sb[:], w_dram[:])
            b_sb = cpool.tile([P, 1], _F32)
            nc.sync.dma_start(b_sb[:], b_dram[:])
            x_sb = cpool.tile([P, NCHUNK, 4 * 4 * NI], _F16)

            o_ts = {}
            for ci, (c0, n) in enumerate(CH):
                # center operand slab for this chunk, loaded just-in-time
                nc.sync.dma_start(x_sb[:, ci, :], xall_d[:, ci, :])
                g_ts = []
                for k in range(K):
                    g_t = gpool.tile([P, NPAIR, NI], _F16)
                    io = (k * NCHUNK + ci) * (NI // 16)
                    nc.gpsimd.dma_gather(
                        g_t[:], xtab_d[:], idx_sb[:, io:io + NI // 16],
                        num_idxs=NI, num_idxs_reg=n,
                        elem_size=ELEM, transpose=True,
                        queue_num=call_no % NQ)
                    call_no += 1
                    g_ts.append(g_t)
                gi = ci % GRP          # position within output group
                for qd in range(4):
                    ps = pspool.tile([P, 4 * NI], _F32)
                    nc.tensor.matmul(
                        ps[:, :], w_sb[:, 0, :],
                        x_sb[:, ci, qd * 4 * NI:(qd + 1) * 4 * NI],
                        start=True, stop=False)
                    for k in range(K):
                        nc.tensor.matmul(
                            ps[:, :], w_sb[:, k + 1, :],
                            g_ts[k][:, qd * 4:qd * 4 + 4, :],
                            start=False, stop=(k == K - 1))
                    for q in range(4):
                        pair = qd * 4 + q
                        if gi == 0:
                            o_ts[pair] = opool.tile([P, GRP * LIVE], _F16,
                                                    name="o_t", tag="o_t")
                        nc.vector.tensor_scalar_add(
                            o_ts[pair][:, gi * LIVE:gi * LIVE + n],
                            ps[:, q * NI:q * NI + n], b_sb[:, :1])
                if gi == GRP - 1 or ci == NCHUNK - 1:
                    g0 = (ci // GRP) * GRP * LIVE
                    gw = (gi + 1) * LIVE
                    for pair in range(NPAIR):
                        nc.sync.dma_start(
                            out_d[pair, :, g0:g0 + gw],
                            o_ts[pair][:, :gw])

    nc.compile()
    return nc


def _run(inputs, trace=False):
    xtab, x_all, idx_w, w_pack, bias2 = _host_prep(
        inputs["x"], inputs["neighbors"], inputs["weight_center"],
        inputs["weight_neighbors"], inputs["bias"])
    nc = _build_program(idx_w, w_pack, bias2)
    in_maps = [{"xtab": xtab[c], "xall": x_all[c]} for c in range(NCORES)]
    res = bass_utils.run_bass_kernel_spmd(
        nc, in_maps, core_ids=list(range(NCORES)), trace=trace)
    outs = np.stack([r["out"] for r in res.results])  # [NC, NPAIR, 128, HPAD]
    outs = outs[:, :, :, :H].astype(np.float32)
    outs = outs.reshape(NCORES, NPAIR, 2, C, H).reshape(B, C, H)
    return np.ascontiguousarray(outs), res


def kernel(x, neighbors, weight_center, weight_neighbors, bias):
    out, _ = _run(dict(x=x, neighbors=neighbors, weight_center=weight_center,
                       weight_neighbors=weight_neighbors, bias=bias))
    return out
